# revision 47
# baseline (speedup 1.0000x reference)
"""ABCDense (ShiftedSteSign 3-estimator binary dense) Trainium2 kernel.

Math:
    xq   = sign(x)                      [N, D]   (+1 for x >= 0)
    beta = mean(|x|, axis=-1)           [N]
    out  = sum_e (xq @ sign(k_e)) * (beta[:,None] * a_e[None,:])

Folded form used here (column scaling commutes with the GEMM):
    W    = sum_e sign(k_e) * (a_e / D)[None,:]        [D, U]  (bf16)
    out  = sum_abs_x[:,None] * (xq @ W)

Distribution: pure data-parallel over the N=32768 token axis across 8
cores (4096 rows each); k/a replicated (no collectives: any cross-core
sync point adds the cores' launch skew to the measured span).

Per-core structure (per 128-token n-tile):
  load x f32 natural -> DVE |x|=max(-x,x) with fused row-sum (beta)
  -> 8 PE transposes (f32, full-rate transpose_mode) into PSUM
  -> ACT evacuates PSUM with func=Sign straight to bf16 xqT (fused
     transpose-evac + quantize)  -> 16 bf16 matmuls vs folded W
  -> DVE PSUM evacuation with fused per-partition beta scale, bf16 out.
The +-1 GEMM is exact in bf16; PSUM accumulates exact integers.
"""

import numpy as np

import concourse.mybir as mybir
from concourse import bacc, tile
from concourse.bass_utils import run_bass_kernel_spmd
from concourse.masks import make_identity

F32 = mybir.dt.float32
BF16 = mybir.dt.bfloat16
AF = mybir.ActivationFunctionType
ALU = mybir.AluOpType

N, D, U, E = 32768, 1024, 1024, 3
NCORES = 8
NS = N // NCORES            # 4096 rows per core
P = 128                     # partitions
DC = D // P                 # 8 d-chunks
NT = NS // P                # 32 n-tiles of 128 tokens
UH = 512                    # u half (one PSUM bank per matmul)


def build_nc():
    nc = bacc.Bacc(
        "TRN2",
        target_bir_lowering=False,
        debug=False,
        num_devices=NCORES,
    )

    # --- DRAM parameters (per-core shapes) ---
    x_d = nc.dram_tensor("x", [NS, D], F32, kind="ExternalInput")
    ks = [nc.dram_tensor(f"k{e}", [D, U], F32, kind="ExternalInput") for e in range(E)]
    as_ = [nc.dram_tensor(f"a{e}", [1, U], F32, kind="ExternalInput") for e in range(E)]
    out_d = nc.dram_tensor("out", [NS, U], BF16, kind="ExternalOutput")

    with tile.TileContext(nc) as tc:
        with (
            tc.tile_pool(name="const", bufs=1) as const,
            tc.tile_pool(name="kstage", bufs=2) as kpool,
            tc.tile_pool(name="xin", bufs=10) as xpool,
            tc.tile_pool(name="scr", bufs=2) as scrpool,
            tc.tile_pool(name="xqp", bufs=18) as xqpool,
            tc.tile_pool(name="osb", bufs=3) as opool,
            tc.tile_pool(name="psT", bufs=4, space="PSUM") as psumT,
            tc.tile_pool(name="psM", bufs=2, space="PSUM") as psumM,
        ):
            # ---------- constants ----------
            ident = const.tile([P, P], F32)
            make_identity(nc, ident[:])

            beta_cols = const.tile([P, NT], F32)

            KCH = 4
            NSLAB = DC // KCH

            # a_e / D as bf16, broadcast across partitions
            a_bcast = []
            for e in range(E):
                a_f = const.tile([1, U], F32, tag=f"a_f{e}")
                nc.sync.dma_start(out=a_f[0:1, :], in_=as_[e][:, :])
                a_b = const.tile([1, U], BF16, tag=f"a_b{e}")
                nc.vector.tensor_scalar(
                    a_b[0:1, :], a_f[0:1, :], 1.0 / D, None, op0=ALU.mult
                )
                a_full = const.tile([P, U], BF16, tag=f"a_full{e}")
                nc.gpsimd.partition_broadcast(a_full[:], a_b[0:1, :])
                a_bcast.append(a_full)

            # ---------- W = sum_e sign(k_e) * a_e / D  (bf16, [d-part, u]) ----------
            W = const.tile([P, DC, U], BF16)
            for half in range(NSLAB):
                for e in range(E):
                    ke = kpool.tile([P, KCH, U], F32, tag="ke")
                    nc.sync.dma_start(
                        out=ke[:],
                        in_=ks[e][half * KCH * P:(half + 1) * KCH * P, :].rearrange(
                            "(c p) u -> p c u", p=P
                        ),
                    )
                    s_e = kpool.tile([P, KCH, U], BF16, tag="se")
                    nc.scalar.activation(s_e[:], ke[:], AF.Sign)
                    for cc in range(KCH):
                        c = half * KCH + cc
                        if e == 0:
                            nc.vector.tensor_tensor(
                                W[:, c, :], s_e[:, cc, :], a_bcast[e][:], op=ALU.mult
                            )
                        else:
                            tmp = kpool.tile([P, U], BF16, tag="tmp")
                            nc.vector.tensor_tensor(
                                tmp[:], s_e[:, cc, :], a_bcast[e][:], op=ALU.mult
                            )
                            nc.vector.tensor_tensor(
                                W[:, c, :], W[:, c, :], tmp[:], op=ALU.add
                            )

            # ---------- per-group pipeline (batch transposes away from MMs) ----------
            GSZ = 8                      # n-tiles per group
            for g in range(NT // GSZ):
                tiles = range(g * GSZ, (g + 1) * GSZ)

                xts = {}
                for t in tiles:
                    x_t = xpool.tile([P, D], F32, tag="xt")
                    nc.sync.dma_start(out=x_t[:], in_=x_d[t * P:(t + 1) * P, :])
                    xts[t] = x_t
                    # beta: |x|=max(-x,x) with fused row-sum on DVE (raw sum;
                    # the 1/D of beta is folded into W)
                    scratch = scrpool.tile([P, D], F32, tag="scratch")
                    nc.vector.scalar_tensor_tensor(
                        scratch[:], x_t[:], -1.0, x_t[:],
                        op0=ALU.mult, op1=ALU.max,
                        accum_out=beta_cols[:, t:t + 1],
                    )
                # transpose x (f32) through PSUM, evac with fused Sign -> bf16
                xqTs = {}
                for t in tiles:
                    x_t = xts[t]
                    xqT = xqpool.tile([P, DC, P], BF16, tag="xqT")
                    for half in range(2):
                        psT = psumT.tile([P, 4 * P], F32, tag="psT")
                        for j in range(4):
                            c = 4 * half + j
                            nc.tensor.transpose(
                                psT[:, j * P:(j + 1) * P],
                                x_t[:, c * P:(c + 1) * P],
                                ident[:],
                            )
                        nc.scalar.activation(
                            xqT[:, 4 * half:4 * half + 4, :], psT[:], AF.Sign
                        )
                    xqTs[t] = xqT

                # main GEMM: out[t] = xqT.T @ W (accumulate over d-chunks)
                for t in tiles:
                    xqT = xqTs[t]
                    ps0 = psumM.tile([P, UH], F32, tag="ps0")
                    ps1 = psumM.tile([P, UH], F32, tag="ps1")
                    ps = [ps0, ps1]
                    for c in range(DC):
                        for h in range(2):
                            nc.tensor.matmul(
                                ps[h][:],
                                xqT[:, c, :],
                                W[:, c, h * UH:(h + 1) * UH],
                                start=(c == 0), stop=(c == DC - 1),
                            )
                    # evacuation with fused per-partition beta scale (bf16 out)
                    osb = opool.tile([P, U], BF16, tag="osb")
                    bcol = beta_cols[:, t:t + 1]
                    for h in range(2):
                        nc.vector.tensor_scalar(
                            osb[:, h * UH:(h + 1) * UH], ps[h][:], bcol, None,
                            op0=ALU.mult,
                        )
                    nc.sync.dma_start(out=out_d[t * P:(t + 1) * P, :], in_=osb[:])

    nc.compile()
    return nc


_CACHE = {}


def _get_nc():
    if "nc" not in _CACHE:
        _CACHE["nc"] = build_nc()
    return _CACHE["nc"]


def make_in_maps(x, k0, k1, k2, a0, a1, a2):
    x = np.ascontiguousarray(x, dtype=np.float32)
    ks = [np.ascontiguousarray(k, dtype=np.float32) for k in (k0, k1, k2)]
    as_ = [np.ascontiguousarray(a, dtype=np.float32).reshape(1, U) for a in (a0, a1, a2)]
    in_maps = []
    for i in range(NCORES):
        shard = np.ascontiguousarray(x[i * NS:(i + 1) * NS])
        in_maps.append({
            "x": shard,
            **{f"k{e}": ks[e] for e in range(E)},
            **{f"a{e}": as_[e] for e in range(E)},
        })
    return in_maps


def run_sharded(x, k0, k1, k2, a0, a1, a2, trace=False, **kw):
    nc = _get_nc()
    in_maps = make_in_maps(x, k0, k1, k2, a0, a1, a2)
    res = run_bass_kernel_spmd(nc, in_maps, list(range(NCORES)), trace=trace, **kw)
    out = np.concatenate(
        [np.asarray(res.results[i]["out"]).astype(np.float32) for i in range(NCORES)],
        axis=0,
    )
    return out, res


def kernel(x, k0, k1, k2, a0, a1, a2):
    out, _ = run_sharded(x, k0, k1, k2, a0, a1, a2, trace=False)
    return out


# revision 48
# speedup vs baseline: 1.1392x; 1.1392x over previous
"""ABCDense (ShiftedSteSign 3-estimator binary dense) Trainium2 kernel.

Math:
    xq   = sign(x)                      [N, D]   (+1 for x >= 0)
    beta = mean(|x|, axis=-1)           [N]
    out  = sum_e (xq @ sign(k_e)) * (beta[:,None] * a_e[None,:])

Folded form used here (column scaling commutes with the GEMM):
    W    = sum_e sign(k_e) * (a_e / D)[None,:]        [D, U]  (bf16)
    out  = sum_abs_x[:,None] * (xq @ W)

Distribution: pure data-parallel over the N=32768 token axis across 8
cores (4096 rows each); k/a replicated (no collectives: any cross-core
sync point adds the cores' launch skew to the measured span).

Per-core structure (per 128-token n-tile):
  load x f32 natural -> DVE |x|=max(-x,x) with fused row-sum (beta)
  -> 8 PE transposes (f32, full-rate transpose_mode) into PSUM
  -> ACT evacuates PSUM with func=Sign straight to bf16 xqT (fused
     transpose-evac + quantize)  -> 16 bf16 matmuls vs folded W
  -> DVE PSUM evacuation with fused per-partition beta scale, bf16 out.
The +-1 GEMM is exact in bf16; PSUM accumulates exact integers.
"""

import numpy as np

import concourse.mybir as mybir
from concourse import bacc, tile
from concourse.bass_utils import run_bass_kernel_spmd
from concourse.masks import make_identity

F32 = mybir.dt.float32
BF16 = mybir.dt.bfloat16
AF = mybir.ActivationFunctionType
ALU = mybir.AluOpType

N, D, U, E = 32768, 1024, 1024, 3
NCORES = 8
NS = N // NCORES            # 4096 rows per core
P = 128                     # partitions
DC = D // P                 # 8 d-chunks
NT = NS // P                # 32 n-tiles of 128 tokens
UH = 512                    # u half (one PSUM bank per matmul)


def build_nc():
    nc = bacc.Bacc(
        "TRN2",
        target_bir_lowering=False,
        debug=False,
        num_devices=NCORES,
    )

    # --- DRAM parameters (per-core shapes) ---
    x_d = nc.dram_tensor("x", [NS, D], F32, kind="ExternalInput")
    ks = [nc.dram_tensor(f"k{e}", [D, U], F32, kind="ExternalInput") for e in range(E)]
    as_ = [nc.dram_tensor(f"a{e}", [1, U], F32, kind="ExternalInput") for e in range(E)]
    out_d = nc.dram_tensor("out", [NS, U], BF16, kind="ExternalOutput")

    with tile.TileContext(nc) as tc:
        with (
            tc.tile_pool(name="const", bufs=1) as const,
            tc.tile_pool(name="kstage", bufs=3) as kpool,
            tc.tile_pool(name="xin", bufs=10) as xpool,
            tc.tile_pool(name="scr", bufs=2) as scrpool,
            tc.tile_pool(name="xqp", bufs=18) as xqpool,
            tc.tile_pool(name="osb", bufs=3) as opool,
            tc.tile_pool(name="psT", bufs=4, space="PSUM") as psumT,
            tc.tile_pool(name="psM", bufs=2, space="PSUM") as psumM,
        ):
            # ---------- constants ----------
            ident = const.tile([P, P], F32)
            make_identity(nc, ident[:])

            beta_cols = const.tile([P, NT], F32)

            KCH = 2
            NSLAB = DC // KCH

            # prefetch the first x tiles so the PE has transpose work
            # while the k load owns the HBM
            xpre = {}
            for t in range(2):
                x_t = xpool.tile([P, D], F32, tag="xt")
                nc.sync.dma_start(out=x_t[:], in_=x_d[t * P:(t + 1) * P, :])
                xpre[t] = x_t

            # a_e / D as bf16, broadcast across partitions
            a_bcast = []
            for e in range(E):
                a_f = const.tile([1, U], F32, tag=f"a_f{e}")
                nc.sync.dma_start(out=a_f[0:1, :], in_=as_[e][:, :])
                a_b = const.tile([1, U], BF16, tag=f"a_b{e}")
                nc.vector.tensor_scalar(
                    a_b[0:1, :], a_f[0:1, :], 1.0 / D, None, op0=ALU.mult
                )
                a_full = const.tile([P, U], BF16, tag=f"a_full{e}")
                nc.gpsimd.partition_broadcast(a_full[:], a_b[0:1, :])
                a_bcast.append(a_full)

            # ---------- W = sum_e sign(k_e) * a_e / D  (bf16, [d-part, u]) ----------
            W = const.tile([P, DC, U], BF16)
            for half in range(NSLAB):
                for e in range(E):
                    ke = kpool.tile([P, KCH, U], F32, tag="ke")
                    nc.sync.dma_start(
                        out=ke[:],
                        in_=ks[e][half * KCH * P:(half + 1) * KCH * P, :].rearrange(
                            "(c p) u -> p c u", p=P
                        ),
                    )
                    s_e = kpool.tile([P, KCH, U], BF16, tag="se")
                    nc.scalar.activation(s_e[:], ke[:], AF.Sign)
                    for cc in range(KCH):
                        c = half * KCH + cc
                        if e == 0:
                            nc.vector.tensor_tensor(
                                W[:, c, :], s_e[:, cc, :], a_bcast[e][:], op=ALU.mult
                            )
                        else:
                            tmp = kpool.tile([P, U], BF16, tag="tmp")
                            nc.vector.tensor_tensor(
                                tmp[:], s_e[:, cc, :], a_bcast[e][:], op=ALU.mult
                            )
                            nc.vector.tensor_tensor(
                                W[:, c, :], W[:, c, :], tmp[:], op=ALU.add
                            )

            # ---------- per-group pipeline (batch transposes away from MMs) ----------
            GSZ = 8                      # n-tiles per group
            for g in range(NT // GSZ):
                tiles = range(g * GSZ, (g + 1) * GSZ)

                xts = {}
                for t in tiles:
                    if t in xpre:
                        x_t = xpre[t]
                    else:
                        x_t = xpool.tile([P, D], F32, tag="xt")
                        nc.sync.dma_start(out=x_t[:], in_=x_d[t * P:(t + 1) * P, :])
                    xts[t] = x_t
                    # beta: |x|=max(-x,x) with fused row-sum on DVE (raw sum;
                    # the 1/D of beta is folded into W)
                    scratch = scrpool.tile([P, D], F32, tag="scratch")
                    nc.vector.scalar_tensor_tensor(
                        scratch[:], x_t[:], -1.0, x_t[:],
                        op0=ALU.mult, op1=ALU.max,
                        accum_out=beta_cols[:, t:t + 1],
                    )
                # transpose x (f32) through PSUM, evac with fused Sign -> bf16
                xqTs = {}
                for t in tiles:
                    x_t = xts[t]
                    xqT = xqpool.tile([P, DC, P], BF16, tag="xqT")
                    for half in range(2):
                        psT = psumT.tile([P, 4 * P], F32, tag="psT")
                        for j in range(4):
                            c = 4 * half + j
                            nc.tensor.transpose(
                                psT[:, j * P:(j + 1) * P],
                                x_t[:, c * P:(c + 1) * P],
                                ident[:],
                            )
                        nc.scalar.activation(
                            xqT[:, 4 * half:4 * half + 4, :], psT[:], AF.Sign
                        )
                    xqTs[t] = xqT

                # main GEMM: out[t] = xqT.T @ W (accumulate over d-chunks)
                for t in tiles:
                    xqT = xqTs[t]
                    ps0 = psumM.tile([P, UH], F32, tag="ps0")
                    ps1 = psumM.tile([P, UH], F32, tag="ps1")
                    ps = [ps0, ps1]
                    for c in range(DC):
                        for h in range(2):
                            nc.tensor.matmul(
                                ps[h][:],
                                xqT[:, c, :],
                                W[:, c, h * UH:(h + 1) * UH],
                                start=(c == 0), stop=(c == DC - 1),
                            )
                    # evacuation with fused per-partition beta scale (bf16 out)
                    osb = opool.tile([P, U], BF16, tag="osb")
                    bcol = beta_cols[:, t:t + 1]
                    for h in range(2):
                        nc.vector.tensor_scalar(
                            osb[:, h * UH:(h + 1) * UH], ps[h][:], bcol, None,
                            op0=ALU.mult,
                        )
                    nc.sync.dma_start(out=out_d[t * P:(t + 1) * P, :], in_=osb[:])

    nc.compile()
    return nc


_CACHE = {}


def _get_nc():
    if "nc" not in _CACHE:
        _CACHE["nc"] = build_nc()
    return _CACHE["nc"]


def make_in_maps(x, k0, k1, k2, a0, a1, a2):
    x = np.ascontiguousarray(x, dtype=np.float32)
    ks = [np.ascontiguousarray(k, dtype=np.float32) for k in (k0, k1, k2)]
    as_ = [np.ascontiguousarray(a, dtype=np.float32).reshape(1, U) for a in (a0, a1, a2)]
    in_maps = []
    for i in range(NCORES):
        shard = np.ascontiguousarray(x[i * NS:(i + 1) * NS])
        in_maps.append({
            "x": shard,
            **{f"k{e}": ks[e] for e in range(E)},
            **{f"a{e}": as_[e] for e in range(E)},
        })
    return in_maps


def run_sharded(x, k0, k1, k2, a0, a1, a2, trace=False, **kw):
    nc = _get_nc()
    in_maps = make_in_maps(x, k0, k1, k2, a0, a1, a2)
    res = run_bass_kernel_spmd(nc, in_maps, list(range(NCORES)), trace=trace, **kw)
    out = np.concatenate(
        [np.asarray(res.results[i]["out"]).astype(np.float32) for i in range(NCORES)],
        axis=0,
    )
    return out, res


def kernel(x, k0, k1, k2, a0, a1, a2):
    out, _ = run_sharded(x, k0, k1, k2, a0, a1, a2, trace=False)
    return out
